# revision 1
# baseline (speedup 1.0000x reference)
"""AdaptiveGCN forward on 8 Trainium2 NeuronCores.

Strategy (per sharding hint): data-parallel over batch N=64 -> 8 shards of 8
samples. Weights (PA/alpha/conv weights) are replicated. Each NeuronCore
computes the full AdaptiveGCN block for its batch shard; results are
concatenated on the host. Forward-only => no collectives needed.

Math per sample n, subset i (identical to the reference):
    a1 = (Wa_i x)        permuted to [V, O*T]
    a2 = (Wb_i x)        reshaped to [O*T, V]
    att = softmax(a1 @ a2 / (O*T), axis=-2)
    A   = PA_i + alpha * att
    s1  = x_flat @ A
    se  = sigmoid(conv1d(relu(conv1d(mean_v(x), w1_i) + b1_i), w2_i) + b2_i)
    y  += Wd_i (s1 * (1 + se))  + bd_i

The attention logits are computed via the algebraic identity
    a1 @ a2 = sum_t x_t^T M x_t + sp 1^T + 1 sq^T + T*(ba.bb),
    M = Wa^T Wb,  sp = Xs^T (Wa^T bb), sq = Xs^T (Wb^T ba), Xs = sum_t x_t
which avoids materializing the [O*T, V] tensors (memory regime).
"""

import numpy as np

N, C, T, V = 64, 64, 300, 25
O, S, INTER, K = 64, 3, 16, 9
N_CORES = 8
SHARD = N // N_CORES

_COMPILED = {}


def _se_gate(x_mean, w1, b1, w2, b2):
    """TemporalSE: x_mean [n,C,T] -> gate [n,1,T]. jnp impl, K=9 'same' conv."""
    import jax
    import jax.numpy as jnp

    pad = (K - 1) // 2
    y = jax.lax.conv_general_dilated(
        x_mean, w1, window_strides=(1,), padding=[(pad, pad)],
        dimension_numbers=("NCH", "OIH", "NCH"))
    y = jax.nn.relu(y + b1[None, :, None])
    y = jax.lax.conv_general_dilated(
        y, w2, window_strides=(1,), padding=[(pad, pad)],
        dimension_numbers=("NCH", "OIH", "NCH"))
    return jax.nn.sigmoid(y + b2[None, :, None])


def _shard_fn(x, PA, alpha, wa, ba, wb, bb, w1, b1, w2, b2, wd, bd):
    """Forward for one batch shard x: [SHARD, C, T, V] -> [SHARD, O, T, V]."""
    import jax.numpy as jnp
    import jax

    n = x.shape[0]
    scale = O * T
    se_in = x.mean(-1)                       # [n, C, T]
    x_flat = x.reshape(n, C * T, V)
    Xs = x.sum(2)                            # [n, C, V]

    y = jnp.zeros((n, O, T, V), dtype=jnp.float32)
    for i in range(S):
        M = wa[i].T @ wb[i]                  # [C, C]
        p = wa[i].T @ bb[i]                  # [C]
        q = wb[i].T @ ba[i]                  # [C]
        r = T * jnp.dot(ba[i], bb[i])
        # G[n,v,v'] = sum_{c,c',t} x[n,c,t,v] M[c,c'] x[n,c',t,v']
        Z = jnp.einsum("cd,ndtv->nctv", M, x)
        G = jnp.einsum("nctv,nctw->nvw", x, Z)
        logits = (G + jnp.einsum("c,ncv->nv", p, Xs)[:, :, None]
                  + jnp.einsum("c,ncv->nv", q, Xs)[:, None, :] + r) / scale
        att = jax.nn.softmax(logits, axis=1)
        A = PA[i][None] + att * alpha[0]     # [n, V, V]
        s1 = jnp.matmul(x_flat, A).reshape(n, C, T, V)
        se = _se_gate(se_in, w1[i], b1[i], w2[i], b2[i])  # [n,1,T]
        t1 = s1 * (1.0 + se[..., None])
        y = y + jnp.einsum("oc,nctv->notv", wd[i], t1) + bd[i][None, :, None, None]
    return y


def _get_compiled(dev):
    import jax
    key = id(dev)
    if key not in _COMPILED:
        _COMPILED[key] = jax.jit(_shard_fn, device=dev)
    return _COMPILED[key]


def kernel(**inputs):
    import jax

    x = np.asarray(inputs["x"], dtype=np.float32)
    weights = {k: np.asarray(v, dtype=np.float32) for k, v in inputs.items()
               if k != "x"}

    devs = jax.devices()[:N_CORES]
    futures = []
    for d_idx, dev in enumerate(devs):
        shard = x[d_idx * SHARD:(d_idx + 1) * SHARD]
        fn = _get_compiled(dev)
        args = [jax.device_put(shard, dev)] + [
            jax.device_put(weights[k], dev)
            for k in ("PA", "alpha", "wa", "ba", "wb", "bb",
                      "w1", "b1", "w2", "b2", "wd", "bd")]
        futures.append(fn(*args))
    out = np.concatenate([np.asarray(f) for f in futures], axis=0)
    return out.astype(np.float32)


if __name__ == "__main__":
    import jax
    rng = np.random.default_rng(0)
    print(jax.devices())


# revision 3
# speedup vs baseline: 1.2816x; 1.2816x over previous
"""AdaptiveGCN forward on 8 Trainium2 NeuronCores.

Strategy (per sharding hint): data-parallel over batch N=64 -> 8 shards of 8
samples. Weights (PA/alpha/conv weights) are replicated. Each NeuronCore
computes the full AdaptiveGCN block for its batch shard; results are
concatenated on the host. Forward-only => no collectives needed.

Math per sample n, subset i (identical to the reference):
    a1 = (Wa_i x)        permuted to [V, O*T]
    a2 = (Wb_i x)        reshaped to [O*T, V]
    att = softmax(a1 @ a2 / (O*T), axis=-2)
    A   = PA_i + alpha * att
    s1  = x_flat @ A
    se  = sigmoid(conv1d(relu(conv1d(mean_v(x), w1_i) + b1_i), w2_i) + b2_i)
    y  += Wd_i (s1 * (1 + se))  + bd_i

The attention logits are computed via the algebraic identity
    a1 @ a2 = sum_t x_t^T M x_t + sp 1^T + 1 sq^T + T*(ba.bb),
    M = Wa^T Wb,  sp = Xs^T (Wa^T bb), sq = Xs^T (Wb^T ba), Xs = sum_t x_t
which avoids materializing the [O*T, V] tensors (memory regime).
"""

import os
import numpy as np
from concurrent.futures import ThreadPoolExecutor

N, C, T, V = 64, 64, 300, 25
O, S, INTER, K = 64, 3, 16, 9
N_CORES = 8
SHARD = N // N_CORES

_COMPILED = {}


def _setup_cache():
    try:
        import jax
        cache_dir = os.environ.get("JAX_COMPILATION_CACHE_DIR",
                                   "/tmp/jax_kernel_cache")
        os.makedirs(cache_dir, exist_ok=True)
        jax.config.update("jax_compilation_cache_dir", cache_dir)
        jax.config.update("jax_persistent_cache_min_entry_size_bytes", -1)
        jax.config.update("jax_persistent_cache_min_compile_time_secs", 0)
    except Exception:
        pass


_setup_cache()


def _se_gate(x_mean, w1, b1, w2, b2):
    """TemporalSE: x_mean [n,C,T] -> gate [n,1,T]. jnp impl, K=9 'same' conv."""
    import jax
    import jax.numpy as jnp

    pad = (K - 1) // 2
    y = jax.lax.conv_general_dilated(
        x_mean, w1, window_strides=(1,), padding=[(pad, pad)],
        dimension_numbers=("NCH", "OIH", "NCH"))
    y = jax.nn.relu(y + b1[None, :, None])
    y = jax.lax.conv_general_dilated(
        y, w2, window_strides=(1,), padding=[(pad, pad)],
        dimension_numbers=("NCH", "OIH", "NCH"))
    return jax.nn.sigmoid(y + b2[None, :, None])


def _shard_fn(x, PA, alpha, wa, ba, wb, bb, w1, b1, w2, b2, wd, bd):
    """Forward for one batch shard x: [SHARD, C, T, V] -> [SHARD, O, T, V]."""
    import jax.numpy as jnp
    import jax

    n = x.shape[0]
    scale = O * T
    se_in = x.mean(-1)                       # [n, C, T]
    x_flat = x.reshape(n, C * T, V)
    Xs = x.sum(2)                            # [n, C, V]

    y = jnp.zeros((n, O, T, V), dtype=jnp.float32)
    for i in range(S):
        M = wa[i].T @ wb[i]                  # [C, C]
        p = wa[i].T @ bb[i]                  # [C]
        q = wb[i].T @ ba[i]                  # [C]
        r = T * jnp.dot(ba[i], bb[i])
        # G[n,v,v'] = sum_{c,c',t} x[n,c,t,v] M[c,c'] x[n,c',t,v']
        Z = jnp.einsum("cd,ndtv->nctv", M, x)
        G = jnp.einsum("nctv,nctw->nvw", x, Z)
        logits = (G + jnp.einsum("c,ncv->nv", p, Xs)[:, :, None]
                  + jnp.einsum("c,ncv->nv", q, Xs)[:, None, :] + r) / scale
        att = jax.nn.softmax(logits, axis=1)
        A = PA[i][None] + att * alpha[0]     # [n, V, V]
        s1 = jnp.matmul(x_flat, A).reshape(n, C, T, V)
        se = _se_gate(se_in, w1[i], b1[i], w2[i], b2[i])  # [n,1,T]
        t1 = s1 * (1.0 + se[..., None])
        y = y + jnp.einsum("oc,nctv->notv", wd[i], t1) + bd[i][None, :, None, None]
    return y


def _get_compiled(dev):
    import jax
    key = id(dev)
    if key not in _COMPILED:
        _COMPILED[key] = jax.jit(_shard_fn, device=dev)
    return _COMPILED[key]


def kernel(**inputs):
    import jax

    x = np.asarray(inputs["x"], dtype=np.float32)
    weights = {k: np.asarray(v, dtype=np.float32) for k, v in inputs.items()
               if k != "x"}

    devs = jax.devices()[:N_CORES]
    wkeys = ("PA", "alpha", "wa", "ba", "wb", "bb",
             "w1", "b1", "w2", "b2", "wd", "bd")

    def run_shard(d_idx):
        dev = devs[d_idx]
        shard = x[d_idx * SHARD:(d_idx + 1) * SHARD]
        fn = _get_compiled(dev)
        args = [jax.device_put(shard, dev)] + [
            jax.device_put(weights[k], dev) for k in wkeys]
        res = fn(*args)
        return np.asarray(res)

    with ThreadPoolExecutor(max_workers=N_CORES) as ex:
        parts = list(ex.map(run_shard, range(N_CORES)))
    out = np.concatenate(parts, axis=0)
    return out.astype(np.float32)


if __name__ == "__main__":
    import jax
    rng = np.random.default_rng(0)
    print(jax.devices())
